# revision 58
# baseline (speedup 1.0000x reference)
"""Trainium2 Bass kernel for local (windowed causal) self-attention.

Problem: B=2, T=2048, C=1024, 16 heads x 64 dim, local window 256.
Sharding: T-sharding. 8 cores = 2 batches x 4 chunks of 512 tokens.
Each core receives its 512-token chunk plus a 256-token left halo of x
(pre-transposed to [C, TQ] on the host, zero-padded for chunk 0),
computes QKV / banded attention / output projection for its own rows,
and writes a disjoint [512, 1024] slice of the output. No collectives;
the host concatenates the 8 slices.

Self-contained: hardcodes all shapes; no reads of /root/problem/*.
"""

import os

os.environ.setdefault("MYCRO_LOCAL_CACHE", "1")

import numpy as np

# ---------------------------------------------------------------- constants
B, T, C = 2, 2048, 1024
H, D = 16, 64
WIN = 256                      # local attention context
NCORES = 8
CHUNK = 512                    # queries per core
HALO = 256                     # left halo (== WIN)
TQ = CHUNK + HALO              # 768 x rows per core
P = 128

NQT = CHUNK // P               # 4 query tiles per core
NKT = TQ // P                  # 6 key tiles per core

# Masked (kt, qt) blocks, all handled as multiplicative 0/1 masks on the
# vector engine post-exp (safe: halo x is host-zeroed, so even "invalid"
# scores are exactly 0 and exp never overflows). kt 1..3 have two masked
# blocks; they are adjacent in the slab for kt 1 only.
MASK_BLOCKS = [(0, 0), (1, 0), (1, 1), (2, 0), (2, 2),
               (3, 1), (3, 3), (4, 2), (5, 3)]

# Matmul operand dtypes: "bf16" or "f32r".
SCORE_DT = os.environ.get("KERNEL_SCORE_DT", "bf16")
VALUE_DT = os.environ.get("KERNEL_VALUE_DT", "bf16")
Y_BF16 = os.environ.get("KERNEL_Y_BF16", "1") == "1"
N_WARM = int(os.environ.get("KERNEL_WARM", "8"))
# Q/K projections in fp8e4m3 with DoubleRow (2x PE throughput). Softmax
# normalization absorbs the score-path quantization (rel err ~1.4e-2 vs the
# 2e-2 gate); V stays bf16 (V errors pass straight through to the output).
QK_FP8 = os.environ.get("KERNEL_QK_FP8", "1") == "1"

_MODS = {}                     # cached compiled Bass modules


def _np_dt(name):
    if name == "bf16":
        import ml_dtypes
        return np.dtype(ml_dtypes.bfloat16)
    return np.dtype(np.float32)


# ------------------------------------------------------------- bass builder
def _build_module(zero_bias):
    import concourse.bacc as bacc
    import concourse.mybir as mybir
    import concourse.tile as tile
    from concourse.masks import make_identity
    from contextlib import ExitStack

    F32 = mybir.dt.float32
    BF16 = mybir.dt.bfloat16
    SDT = BF16 if SCORE_DT == "bf16" else mybir.dt.float32r
    VDT = BF16 if VALUE_DT == "bf16" else mybir.dt.float32r
    YDT = BF16 if Y_BF16 else F32

    nc = bacc.Bacc(
        "TRN2",
        target_bir_lowering=False,
        debug=False,
        enable_asserts=False,
        num_devices=NCORES,
    )

    XDT = SDT if SCORE_DT == "bf16" else F32
    F8 = mybir.dt.float8e4
    # x^T is prepared on the host: [C, TQ]
    xh = nc.dram_tensor("xh", [C, TQ], XDT, kind="ExternalInput").ap()
    wa = nc.dram_tensor("wa", [C, 3 * C], SDT, kind="ExternalInput").ap()
    if QK_FP8:
        # fp8 copies of x^T and W_attn[:, :2C] for the Q/K projections
        xh8 = nc.dram_tensor("xh8", [C, TQ], F8, kind="ExternalInput").ap()
        w8 = nc.dram_tensor("w8", [C, 2 * C], F8, kind="ExternalInput").ap()
    ba = nc.dram_tensor("ba", [3 * C], F32, kind="ExternalInput").ap()
    wp = nc.dram_tensor("wp", [C, C], VDT, kind="ExternalInput").ap()
    bp = nc.dram_tensor("bp", [C], F32, kind="ExternalInput").ap()
    # multiplicative (0/1) mask tiles for MASK_BLOCKS: [128k, 9, 128q]
    mk = nc.dram_tensor("mk", [P, 9, P], VDT, kind="ExternalInput").ap()
    y = nc.dram_tensor("y", [CHUNK, C], YDT, kind="ExternalOutput").ap()

    Exp = mybir.ActivationFunctionType.Exp
    Ident = mybir.ActivationFunctionType.Identity
    ADD = mybir.AluOpType.add
    MUL = mybir.AluOpType.mult

    with tile.TileContext(nc) as tc, ExitStack() as ctx:
        const = ctx.enter_context(tc.tile_pool(name="const", bufs=1))
        xload = ctx.enter_context(tc.tile_pool(name="xload", bufs=2))
        big = ctx.enter_context(tc.tile_pool(name="big", bufs=1))
        wpool = ctx.enter_context(tc.tile_pool(name="wpool", bufs=3))
        slabp = ctx.enter_context(tc.tile_pool(name="slabp", bufs=4))
        small = ctx.enter_context(tc.tile_pool(name="small", bufs=16))
        yout = ctx.enter_context(tc.tile_pool(name="yout", bufs=4))
        # PSUM: 8 banks of 2KB. ps512 x3 (QKV/proj + K), spool x3 (scores),
        # smallp x2 (AV accum + pair transposes, interleaved allocations).
        ps512 = ctx.enter_context(tc.tile_pool(name="ps512", bufs=3, space="PSUM"))
        spool = ctx.enter_context(tc.tile_pool(name="spool", bufs=3, space="PSUM"))
        smallp = ctx.enter_context(tc.tile_pool(name="smallp", bufs=2, space="PSUM"))

        # ---------------- constants
        ident = const.tile([P, P], F32)
        make_identity(nc, ident)
        if VALUE_DT == "bf16":
            identv = const.tile([P, P], BF16)
            make_identity(nc, identv)
            PAIR_DT = BF16
        else:
            identv = ident
            PAIR_DT = F32

        if not zero_bias:
            bqk = const.tile([P, 16], F32)      # b_attn[:2048] as [128, jt]
            with nc.allow_non_contiguous_dma(reason="tiny bias rearrange"):
                nc.sync.dma_start(
                    bqk, ba[: 2 * C].rearrange("(j p) -> p j", p=P))
            bv_row = xload.tile([1, C], F32, tag="brow")
            nc.sync.dma_start(bv_row, ba[None, 2 * C:])
            bv_b = const.tile([P, C], F32)
            nc.gpsimd.partition_broadcast(bv_b, bv_row)
            bp_row = xload.tile([1, C], F32, tag="brow")
            nc.sync.dma_start(bp_row, bp[None, :])
            bp_b = const.tile([P, C], F32)
            nc.gpsimd.partition_broadcast(bp_b, bp_row)

        # PE warm-up: dense dummy matmuls while the first DMAs land, so the
        # p-state ramp reaches full clock before real matmuls start.
        warm = const.tile([P, 512], BF16)
        nc.vector.memset(warm, 0.0)
        for wi in range(N_WARM):
            wps = ps512.tile([P, 512], F32, tag="ps512", name=f"wps{wi}")
            nc.tensor.matmul(wps, warm[:, :P], warm, start=True, stop=True)

        # ---------------- high-priority DMAs: Q weights (group 0) + own x^T
        # One DMA per logical group (rearranged AP): dma_start issue time on
        # the sync queue is ~0.6us each, so merging is critical for the head.
        xT = big.tile([P, C // P, TQ], SDT, tag="xT")

        def wgroup(src_cols, split=False, fp8=False):
            dt_ = F8 if fp8 else SDT
            base = w8 if fp8 else wa
            wt = wpool.tile([P, C // P, 512], dt_, tag="wchunk")
            src = base[:, src_cols].rearrange("(ct p) j -> p ct j", p=P)
            if split:
                nc.sync.dma_start(wt[:, :4], src[:, :4])
                nc.sync.dma_start(wt[:, 4:], src[:, 4:])
            else:
                nc.sync.dma_start(wt, src)
            return wt

        w_q0 = wgroup(slice(0, 512), split=True, fp8=QK_FP8)
        if QK_FP8:
            xT8 = big.tile([P, C // P, TQ], F8, tag="xT8")
            x8src = xh8.rearrange("(ct p) t -> p ct t", p=P)
            nc.sync.dma_start(xT8[:, :, HALO:TQ], x8src[:, :, HALO:TQ])
            # bf16 x^T next: the V projection needs it right after the
            # (short, fp8) Q phase
            xsrc = xh.rearrange("(ct p) t -> p ct t", p=P)
            nc.sync.dma_start(xT[:, :4], xsrc[:, :4])
            nc.sync.dma_start(xT[:, 4:], xsrc[:, 4:])
        else:
            xsrc = xh[:, HALO:TQ].rearrange("(ct p) t -> p ct t", p=P)
            nc.sync.dma_start(xT[:, :4, HALO:TQ], xsrc[:, :4])
            nc.sync.dma_start(xT[:, 4:, HALO:TQ], xsrc[:, 4:])

        masks = const.tile([P, 9, P], VDT)
        nc.sync.dma_start(masks, mk)

        w_q1 = wgroup(slice(512, 1024), fp8=QK_FP8)
        if QK_FP8:
            nc.sync.dma_start(xT8[:, :, 0:HALO], x8src[:, :, 0:HALO])
        else:
            nc.sync.dma_start(                   # halo columns of x^T
                xT[:, :, 0:HALO],
                xh[:, 0:HALO].rearrange("(ct p) t -> p ct t", p=P))
        w_q = [w_q0, w_q1]

        # ---------------- QKV
        # Q^T [128j, jt, 512t(own)]  /  K^T [128j, jt, 768t]
        QT = big.tile([P, 8, CHUNK], SDT, tag="QT")
        KT = big.tile([P, 8, TQ], SDT, tag="KT")
        # V natural + ones columns: [128t, tt, head, D+2]
        VS = big.tile([P, NKT, H, D + 2], VDT, tag="VS")
        ones_h = const.tile([P, NKT * H], F32)
        nc.gpsimd.memset(ones_h, 1.0)
        nc.vector.tensor_copy(
            VS[:, :, :, D], ones_h.rearrange("p (t h) -> p t h", h=H))
        nc.vector.tensor_copy(
            VS[:, :, :, D + 1], ones_h.rearrange("p (t h) -> p t h", h=H))

        DR = mybir.MatmulPerfMode.DoubleRow

        # --- Q part: lhsT = W_attn[:, j] tile, rhs = xT own rows
        for jg in range(2):                       # 2 groups of 4 j-tiles
            wts = w_q[jg]
            for jl in range(4):
                jt = jg * 4 + jl
                ps = ps512.tile([P, CHUNK], F32, tag="ps512")
                if QK_FP8:
                    for g2 in range(4):           # 4 DoubleRow k-pairs
                        nc.tensor.matmul(
                            ps,
                            wts[:, 2 * g2:2 * g2 + 2, jl * P:(jl + 1) * P],
                            xT8[:, 2 * g2:2 * g2 + 2, HALO:TQ],
                            start=(g2 == 0), stop=(g2 == 3), perf_mode=DR)
                else:
                    for ct in range(C // P):
                        nc.tensor.matmul(
                            ps,
                            wts[:, ct, jl * P:(jl + 1) * P],
                            xT[:, ct, HALO:TQ],
                            start=(ct == 0), stop=(ct == C // P - 1))
                nc.scalar.activation(
                    QT[:, jt, :], ps, Ident, scale=1.0,
                    bias=0.0 if zero_bias else bqk[:, jt:jt + 1])

        # --- V part: lhsT = xT tile, rhs = W_attn[:, 2048+...]
        for vc in range(2):
            wts = wgroup(slice(2 * C + vc * 512, 2 * C + (vc + 1) * 512))
            for tt in range(NKT):
                ps = ps512.tile([P, 512], F32, tag="ps512")
                for ct in range(C // P):
                    nc.tensor.matmul(
                        ps,
                        xT[:, ct, tt * P:(tt + 1) * P],
                        wts[:, ct, :],
                        start=(ct == 0), stop=(ct == C // P - 1))
                if zero_bias:
                    nc.scalar.activation(
                        VS[:, tt, vc * 8:(vc + 1) * 8, 0:D],
                        ps.rearrange("p (h d) -> p h d", d=D),
                        Ident, bias=0.0, scale=1.0)
                else:
                    nc.vector.tensor_tensor(
                        VS[:, tt, vc * 8:(vc + 1) * 8, 0:D],
                        ps.rearrange("p (h d) -> p h d", d=D),
                        bv_b[:, vc * 512:(vc + 1) * 512]
                            .rearrange("p (h d) -> p h d", d=D),
                        ADD)

        # --- K part, with attention head-pairs interleaved so the PE
        # stream stays dense and engines overlap across phases.
        outT = big.tile([P, 8, CHUNK], VDT, tag="outT")  # [c_pair, hp, t]
        scale = 1.0 / np.sqrt(D)

        mask_by_kt = {}
        for i, (kt, qt) in enumerate(MASK_BLOCKS):
            mask_by_kt.setdefault(kt, []).append((i, qt))
        slabs = {}     # (hp, hh) -> slab tile
        pairs = {}     # hp -> [pair tiles]

        def emit_scores_hh(hp, hh):
            p0 = hh * 64
            slab = slabp.tile([P, NKT, 384], VDT, tag="slab",
                              name=f"slab{hp}_{hh}")
            for kt in range(NKT):
                qlo = max(0, kt - 2)
                qhi = min(NQT - 1, kt)
                nq = (qhi - qlo + 1) * P
                ps = spool.tile([P, 384], F32, tag="spool",
                                name=f"st{hp}_{kt}_{hh}")
                nc.tensor.matmul(
                    ps[:, :nq],
                    KT[p0:p0 + 64, hp, kt * P:(kt + 1) * P],
                    QT[p0:p0 + 64, hp, qlo * P: qlo * P + nq],
                    start=True, stop=True)
                nc.scalar.activation(slab[:, kt, :nq], ps[:, :nq], Exp,
                                     bias=0.0, scale=float(scale))
                mis = mask_by_kt.get(kt, ())
                if len(mis) == 2 and mis[1][1] - mis[0][1] == 1:
                    # two adjacent masked blocks (kt==1): one 256-wide op
                    mi, qt = mis[0]
                    qoff = (qt - qlo) * P
                    nc.vector.tensor_tensor(
                        slab[:, kt, qoff:qoff + 2 * P],
                        slab[:, kt, qoff:qoff + 2 * P],
                        masks.rearrange("p a b -> p (a b)")
                             [:, mi * P:(mi + 2) * P], MUL)
                else:
                    for mi, qt in mis:
                        qoff = (qt - qlo) * P
                        nc.vector.tensor_tensor(
                            slab[:, kt, qoff:qoff + P],
                            slab[:, kt, qoff:qoff + P],
                            masks[:, mi, :], MUL)
            slabs[(hp, hh)] = slab

        def emit_av_hh(hp, hh):
            if hh == 0:
                pairs[hp] = [small.tile([P, P], PAIR_DT, tag="pair",
                                        name=f"pair{hp}_{i}")
                             for i in range(NQT)]
            pair = pairs[hp]
            h = 2 * hp + hh
            p0 = hh * 64
            slab = slabs.pop((hp, hh))
            pav = smallp.tile([P, NQT, D + 2], F32, tag="smallp",
                              name=f"pav{hp}_{hh}")
            for qt in range(NQT):
                for i, kt in enumerate(range(qt, qt + 3)):
                    qoff = (qt - max(0, kt - 2)) * P
                    nc.tensor.matmul(
                        pav[:, qt, :],
                        slab[:, kt, qoff:qoff + P],
                        VS[:, kt, h, :],
                        start=(i == 0), stop=(i == 2),
                        skip_group_check=True)
            rcp = small.tile([P, NQT], F32, tag="rcp")
            nc.vector.reciprocal(rcp, pav[:, :, D])
            for qt in range(NQT):
                nc.vector.tensor_scalar_mul(
                    pair[qt][:, p0:p0 + 64], pav[:, qt, 0:D],
                    rcp[:, qt:qt + 1])

        def emit_pair_fin(hp):
            # transpose head-pair outputs into c_in-major layout
            pair = pairs.pop(hp)
            for qg in range(2):
                pt = smallp.tile([P, 2 * P], PAIR_DT, tag="smallp",
                                 name=f"ptr{hp}_{qg}")
                for ql in range(2):
                    nc.tensor.transpose(
                        pt[:, ql * P:(ql + 1) * P], pair[qg * 2 + ql], identv)
                nc.vector.tensor_copy(
                    outT[:, hp, qg * 2 * P:(qg + 1) * 2 * P], pt)

        # 3-stage software pipeline at half-pair (head) granularity: the
        # AV/normalize for head (jt-1, hh) is emitted only once enough
        # independent work (K matmuls, scores) is queued ahead of it that
        # its EXPs have drained; transposes lag a full pair further.
        # K stays bf16: its matmuls usefully fill the latency bubbles of the
        # cross-engine attention chain, so fp8 here buys no wall-clock and
        # costs accuracy.
        for jg in range(2):
            wts = wgroup(slice(C + jg * 512, C + (jg + 1) * 512))
            for jl in range(4):
                jt = jg * 4 + jl
                for half, hw in ((0, 512), (1, 256)):
                    ps = ps512.tile([P, 512], F32, tag="ps512")
                    for ct in range(C // P):
                        nc.tensor.matmul(
                            ps[:, :hw],
                            wts[:, ct, jl * P:(jl + 1) * P],
                            xT[:, ct, half * 512: half * 512 + hw],
                            start=(ct == 0), stop=(ct == C // P - 1))
                    if zero_bias:
                        nc.vector.tensor_copy(
                            KT[:, jt, half * 512: half * 512 + hw],
                            ps[:, :hw])
                    else:
                        nc.vector.tensor_scalar_add(
                            KT[:, jt, half * 512: half * 512 + hw],
                            ps[:, :hw], bqk[:, 8 + jt: 9 + jt])
                if jt >= 2:
                    emit_pair_fin(jt - 2)
                if jt >= 1:
                    emit_av_hh(jt - 1, 0)
                emit_scores_hh(jt, 0)
                if jt >= 1:
                    emit_av_hh(jt - 1, 1)
                emit_scores_hh(jt, 1)
        emit_pair_fin(6)
        emit_av_hh(7, 0)
        emit_av_hh(7, 1)
        emit_pair_fin(7)

        # ---------------- output projection
        for oc in range(2):
            wts = wpool.tile([P, 8, 512], VDT, tag="wchunk")
            nc.sync.dma_start(
                wts, wp[:, oc * 512:(oc + 1) * 512]
                    .rearrange("(hp p) j -> p hp j", p=P))
            for tb in range(NQT):
                ps = ps512.tile([P, 512], F32, tag="ps512")
                for hp in range(8):
                    nc.tensor.matmul(
                        ps,
                        outT[:, hp, tb * P:(tb + 1) * P],
                        wts[:, hp, :],
                        start=(hp == 0), stop=(hp == 7))
                ysb = yout.tile([P, 512], YDT, tag="ysb")
                if zero_bias:
                    nc.scalar.activation(ysb, ps, Ident, bias=0.0, scale=1.0)
                else:
                    nc.vector.tensor_tensor(
                        ysb, ps, bp_b[:, oc * 512:(oc + 1) * 512], ADD)
                nc.sync.dma_start(
                    y[tb * P:(tb + 1) * P, oc * 512:(oc + 1) * 512], ysb)

    nc.compile()
    return nc


def _get_module(zero_bias):
    if zero_bias not in _MODS:
        _MODS[zero_bias] = _build_module(zero_bias)
    return _MODS[zero_bias]


# ------------------------------------------------------------- host helpers
def _mask_tiles(chunk_start: int) -> np.ndarray:
    """[128, 9, 128]: multiplicative (1 valid / 0 invalid) tiles for
    MASK_BLOCKS."""
    out = np.zeros((P, 9, P), np.float32)
    kk = np.arange(P)[:, None]
    qq = np.arange(P)[None, :]

    def valid(kt, qt):
        key_abs = chunk_start - HALO + kt * P + kk
        q_abs = chunk_start + qt * P + qq
        return (key_abs <= q_abs) & (key_abs >= q_abs - WIN) & (key_abs >= 0)

    for i, (kt, qt) in enumerate(MASK_BLOCKS):
        out[:, i, :] = valid(kt, qt).astype(np.float32)
    return out


def _in_maps(x, W_attn, b_attn, W_proj, b_proj):
    import ml_dtypes
    f8 = np.dtype(ml_dtypes.float8_e4m3)
    sdt, vdt = _np_dt(SCORE_DT), _np_dt(VALUE_DT)
    wa = np.ascontiguousarray(np.asarray(W_attn, np.float32).astype(sdt))
    wpp = np.ascontiguousarray(np.asarray(W_proj, np.float32).astype(vdt))
    ba = np.ascontiguousarray(b_attn, np.float32)
    bpp = np.ascontiguousarray(b_proj, np.float32)
    if QK_FP8:
        w8 = np.ascontiguousarray(
            np.asarray(W_attn, np.float32)[:, :2 * C].astype(f8))
    maps = []
    for c in range(NCORES):
        b, k = divmod(c, NCORES // B)
        t0 = k * CHUNK
        xdt = sdt if SCORE_DT == "bf16" else np.dtype(np.float32)
        xhalo = np.zeros((TQ, C), xdt)
        lo = t0 - HALO
        src_lo = max(0, lo)
        xhalo[src_lo - lo:, :] = x[b, src_lo: t0 + CHUNK].astype(xdt)
        xt = np.ascontiguousarray(xhalo.T)            # [C, TQ]
        m = {
            "xh": xt,
            "wa": wa,
            "ba": ba,
            "wp": wpp,
            "bp": bpp,
            "mk": _mask_tiles(t0).astype(vdt),
        }
        if QK_FP8:
            m["xh8"] = np.ascontiguousarray(
                xt.astype(np.float32).astype(f8))
            m["w8"] = w8
        maps.append(m)
    return maps


def _run(inputs, trace=False, trace_kwargs=None):
    from concourse import bass_utils

    zero_bias = (not np.any(inputs["b_attn"])) and \
        (not np.any(inputs["b_proj"]))
    nc = _get_module(zero_bias)
    maps = _in_maps(**inputs)
    res = bass_utils.run_bass_kernel_spmd(
        nc, maps, core_ids=list(range(NCORES)),
        trace=trace, **(trace_kwargs or {}))
    out = np.empty((B, T, C), np.float32)
    for c in range(NCORES):
        b, k = divmod(c, NCORES // B)
        out[b, k * CHUNK:(k + 1) * CHUNK] = \
            np.asarray(res.results[c]["y"], np.float32)
    return out, res


def kernel(x, W_attn, b_attn, W_proj, b_proj):
    inputs = dict(x=np.asarray(x, np.float32), W_attn=W_attn, b_attn=b_attn,
                  W_proj=W_proj, b_proj=b_proj)
    out, _ = _run(inputs)
    return out


# revision 59
# speedup vs baseline: 12536.3179x; 12536.3179x over previous
"""Trainium2 Bass kernel for local (windowed causal) self-attention.

Problem: B=2, T=2048, C=1024, 16 heads x 64 dim, local window 256.
Sharding: T-sharding. 8 cores = 2 batches x 4 chunks of 512 tokens.
Each core receives its 512-token chunk plus a 256-token left halo of x
(pre-transposed to [C, TQ] on the host, zero-padded for chunk 0),
computes QKV / banded attention / output projection for its own rows,
and writes a disjoint [512, 1024] slice of the output. No collectives;
the host concatenates the 8 slices.

Self-contained: hardcodes all shapes; no reads of /root/problem/*.
"""

import os

os.environ.setdefault("MYCRO_LOCAL_CACHE", "1")

import numpy as np

# ---------------------------------------------------------------- constants
B, T, C = 2, 2048, 1024
H, D = 16, 64
WIN = 256                      # local attention context
NCORES = 8
CHUNK = 512                    # queries per core
HALO = 256                     # left halo (== WIN)
TQ = CHUNK + HALO              # 768 x rows per core
P = 128

NQT = CHUNK // P               # 4 query tiles per core
NKT = TQ // P                  # 6 key tiles per core

# Masked (kt, qt) blocks, all handled as multiplicative 0/1 masks on the
# vector engine post-exp (safe: halo x is host-zeroed, so even "invalid"
# scores are exactly 0 and exp never overflows). kt 1..3 have two masked
# blocks; they are adjacent in the slab for kt 1 only.
MASK_BLOCKS = [(0, 0), (1, 0), (1, 1), (2, 0), (2, 2),
               (3, 1), (3, 3), (4, 2), (5, 3)]

# Matmul operand dtypes: "bf16" or "f32r".
SCORE_DT = os.environ.get("KERNEL_SCORE_DT", "bf16")
VALUE_DT = os.environ.get("KERNEL_VALUE_DT", "bf16")
Y_BF16 = os.environ.get("KERNEL_Y_BF16", "1") == "1"
N_WARM = int(os.environ.get("KERNEL_WARM", "8"))
# Q/K projections in fp8e4m3 with DoubleRow (2x PE throughput). Softmax
# normalization absorbs the score-path quantization (rel err ~1.4e-2 vs the
# 2e-2 gate); V stays bf16 (V errors pass straight through to the output).
QK_FP8 = os.environ.get("KERNEL_QK_FP8", "1") == "1"

_MODS = {}                     # cached compiled Bass modules


def _np_dt(name):
    if name == "bf16":
        import ml_dtypes
        return np.dtype(ml_dtypes.bfloat16)
    return np.dtype(np.float32)


# ------------------------------------------------------------- bass builder
def _build_module(zero_bias):
    import concourse.bacc as bacc
    import concourse.mybir as mybir
    import concourse.tile as tile
    from concourse.masks import make_identity
    from contextlib import ExitStack

    F32 = mybir.dt.float32
    BF16 = mybir.dt.bfloat16
    SDT = BF16 if SCORE_DT == "bf16" else mybir.dt.float32r
    VDT = BF16 if VALUE_DT == "bf16" else mybir.dt.float32r
    YDT = BF16 if Y_BF16 else F32

    nc = bacc.Bacc(
        "TRN2",
        target_bir_lowering=False,
        debug=False,
        enable_asserts=False,
        num_devices=NCORES,
    )

    XDT = SDT if SCORE_DT == "bf16" else F32
    F8 = mybir.dt.float8e4
    # x^T is prepared on the host: [C, TQ]
    xh = nc.dram_tensor("xh", [C, TQ], XDT, kind="ExternalInput").ap()
    wa = nc.dram_tensor("wa", [C, 3 * C], SDT, kind="ExternalInput").ap()
    if QK_FP8:
        # fp8 copies of x^T and W_attn[:, :2C] for the Q/K projections
        xh8 = nc.dram_tensor("xh8", [C, TQ], F8, kind="ExternalInput").ap()
        w8 = nc.dram_tensor("w8", [C, 2 * C], F8, kind="ExternalInput").ap()
    ba = nc.dram_tensor("ba", [3 * C], F32, kind="ExternalInput").ap()
    wp = nc.dram_tensor("wp", [C, C], VDT, kind="ExternalInput").ap()
    bp = nc.dram_tensor("bp", [C], F32, kind="ExternalInput").ap()
    # multiplicative (0/1) mask tiles for MASK_BLOCKS: [128k, 9, 128q]
    mk = nc.dram_tensor("mk", [P, 9, P], VDT, kind="ExternalInput").ap()
    y = nc.dram_tensor("y", [CHUNK, C], YDT, kind="ExternalOutput").ap()

    Exp = mybir.ActivationFunctionType.Exp
    Ident = mybir.ActivationFunctionType.Identity
    ADD = mybir.AluOpType.add
    MUL = mybir.AluOpType.mult

    with tile.TileContext(nc) as tc, ExitStack() as ctx:
        const = ctx.enter_context(tc.tile_pool(name="const", bufs=1))
        xload = ctx.enter_context(tc.tile_pool(name="xload", bufs=2))
        big = ctx.enter_context(tc.tile_pool(name="big", bufs=1))
        wpool = ctx.enter_context(tc.tile_pool(name="wpool", bufs=3))
        slabp = ctx.enter_context(tc.tile_pool(name="slabp", bufs=4))
        small = ctx.enter_context(tc.tile_pool(name="small", bufs=16))
        yout = ctx.enter_context(tc.tile_pool(name="yout", bufs=4))
        # PSUM: 8 banks of 2KB. ps512 x3 (QKV/proj + K), spool x3 (scores),
        # smallp x2 (AV accum + pair transposes, interleaved allocations).
        ps512 = ctx.enter_context(tc.tile_pool(name="ps512", bufs=3, space="PSUM"))
        spool = ctx.enter_context(tc.tile_pool(name="spool", bufs=3, space="PSUM"))
        smallp = ctx.enter_context(tc.tile_pool(name="smallp", bufs=2, space="PSUM"))

        # ---------------- constants
        ident = const.tile([P, P], F32)
        make_identity(nc, ident)
        if VALUE_DT == "bf16":
            identv = const.tile([P, P], BF16)
            make_identity(nc, identv)
            PAIR_DT = BF16
        else:
            identv = ident
            PAIR_DT = F32

        if not zero_bias:
            bqk = const.tile([P, 16], F32)      # b_attn[:2048] as [128, jt]
            with nc.allow_non_contiguous_dma(reason="tiny bias rearrange"):
                nc.sync.dma_start(
                    bqk, ba[: 2 * C].rearrange("(j p) -> p j", p=P))
            bv_row = xload.tile([1, C], F32, tag="brow")
            nc.sync.dma_start(bv_row, ba[None, 2 * C:])
            bv_b = const.tile([P, C], F32)
            nc.gpsimd.partition_broadcast(bv_b, bv_row)
            bp_row = xload.tile([1, C], F32, tag="brow")
            nc.sync.dma_start(bp_row, bp[None, :])
            bp_b = const.tile([P, C], F32)
            nc.gpsimd.partition_broadcast(bp_b, bp_row)

        # PE warm-up: dense dummy matmuls while the first DMAs land, so the
        # p-state ramp reaches full clock before real matmuls start.
        warm = const.tile([P, 512], BF16)
        nc.vector.memset(warm, 0.0)
        for wi in range(N_WARM):
            wps = ps512.tile([P, 512], F32, tag="ps512", name=f"wps{wi}")
            nc.tensor.matmul(wps, warm[:, :P], warm, start=True, stop=True)

        # ---------------- high-priority DMAs: Q weights (group 0) + own x^T
        # One DMA per logical group (rearranged AP): dma_start issue time on
        # the sync queue is ~0.6us each, so merging is critical for the head.
        xT = big.tile([P, C // P, TQ], SDT, tag="xT")

        def wgroup(src_cols, split=False, fp8=False):
            dt_ = F8 if fp8 else SDT
            base = w8 if fp8 else wa
            wt = wpool.tile([P, C // P, 512], dt_, tag="wchunk")
            src = base[:, src_cols].rearrange("(ct p) j -> p ct j", p=P)
            if split:
                nc.sync.dma_start(wt[:, :4], src[:, :4])
                nc.sync.dma_start(wt[:, 4:], src[:, 4:])
            else:
                nc.sync.dma_start(wt, src)
            return wt

        w_q0 = wgroup(slice(0, 512), split=True, fp8=QK_FP8)
        if QK_FP8:
            xT8 = big.tile([P, C // P, TQ], F8, tag="xT8")
            x8src = xh8.rearrange("(ct p) t -> p ct t", p=P)
            nc.sync.dma_start(xT8[:, :, HALO:TQ], x8src[:, :, HALO:TQ])
            # bf16 x^T next: the V projection needs it right after the
            # (short, fp8) Q phase
            xsrc = xh.rearrange("(ct p) t -> p ct t", p=P)
            nc.sync.dma_start(xT[:, :4], xsrc[:, :4])
            nc.sync.dma_start(xT[:, 4:], xsrc[:, 4:])
        else:
            xsrc = xh[:, HALO:TQ].rearrange("(ct p) t -> p ct t", p=P)
            nc.sync.dma_start(xT[:, :4, HALO:TQ], xsrc[:, :4])
            nc.sync.dma_start(xT[:, 4:, HALO:TQ], xsrc[:, 4:])

        masks = const.tile([P, 9, P], VDT)
        nc.sync.dma_start(masks, mk)

        w_q1 = wgroup(slice(512, 1024), fp8=QK_FP8)
        if QK_FP8:
            nc.sync.dma_start(xT8[:, :, 0:HALO], x8src[:, :, 0:HALO])
        else:
            nc.sync.dma_start(                   # halo columns of x^T
                xT[:, :, 0:HALO],
                xh[:, 0:HALO].rearrange("(ct p) t -> p ct t", p=P))
        w_q = [w_q0, w_q1]

        # ---------------- QKV
        # Q^T [128j, jt, 512t(own)]  /  K^T [128j, jt, 768t]
        QT = big.tile([P, 8, CHUNK], SDT, tag="QT")
        KT = big.tile([P, 8, TQ], SDT, tag="KT")
        # V natural + ones columns: [128t, tt, head, D+2]
        VS = big.tile([P, NKT, H, D + 2], VDT, tag="VS")
        ones_h = const.tile([P, NKT * H], F32)
        nc.gpsimd.memset(ones_h, 1.0)
        nc.vector.tensor_copy(
            VS[:, :, :, D], ones_h.rearrange("p (t h) -> p t h", h=H))
        nc.vector.tensor_copy(
            VS[:, :, :, D + 1], ones_h.rearrange("p (t h) -> p t h", h=H))

        DR = mybir.MatmulPerfMode.DoubleRow

        # --- Q part: lhsT = W_attn[:, j] tile, rhs = xT own rows
        for jg in range(2):                       # 2 groups of 4 j-tiles
            wts = w_q[jg]
            for jl in range(4):
                jt = jg * 4 + jl
                ps = ps512.tile([P, CHUNK], F32, tag="ps512")
                if QK_FP8:
                    for g2 in range(4):           # 4 DoubleRow k-pairs
                        nc.tensor.matmul(
                            ps,
                            wts[:, 2 * g2:2 * g2 + 2, jl * P:(jl + 1) * P],
                            xT8[:, 2 * g2:2 * g2 + 2, HALO:TQ],
                            start=(g2 == 0), stop=(g2 == 3), perf_mode=DR)
                else:
                    for ct in range(C // P):
                        nc.tensor.matmul(
                            ps,
                            wts[:, ct, jl * P:(jl + 1) * P],
                            xT[:, ct, HALO:TQ],
                            start=(ct == 0), stop=(ct == C // P - 1))
                nc.scalar.activation(
                    QT[:, jt, :], ps, Ident, scale=1.0,
                    bias=0.0 if zero_bias else bqk[:, jt:jt + 1])

        # --- V part: lhsT = xT tile, rhs = W_attn[:, 2048+...]
        for vc in range(2):
            wts = wgroup(slice(2 * C + vc * 512, 2 * C + (vc + 1) * 512))
            for tt in range(NKT):
                ps = ps512.tile([P, 512], F32, tag="ps512")
                for ct in range(C // P):
                    nc.tensor.matmul(
                        ps,
                        xT[:, ct, tt * P:(tt + 1) * P],
                        wts[:, ct, :],
                        start=(ct == 0), stop=(ct == C // P - 1))
                if zero_bias:
                    nc.scalar.activation(
                        VS[:, tt, vc * 8:(vc + 1) * 8, 0:D],
                        ps.rearrange("p (h d) -> p h d", d=D),
                        Ident, bias=0.0, scale=1.0)
                else:
                    nc.vector.tensor_tensor(
                        VS[:, tt, vc * 8:(vc + 1) * 8, 0:D],
                        ps.rearrange("p (h d) -> p h d", d=D),
                        bv_b[:, vc * 512:(vc + 1) * 512]
                            .rearrange("p (h d) -> p h d", d=D),
                        ADD)

        # --- K part, with attention head-pairs interleaved so the PE
        # stream stays dense and engines overlap across phases.
        outT = big.tile([P, 8, CHUNK], VDT, tag="outT")  # [c_pair, hp, t]
        scale = 1.0 / np.sqrt(D)

        mask_by_kt = {}
        for i, (kt, qt) in enumerate(MASK_BLOCKS):
            mask_by_kt.setdefault(kt, []).append((i, qt))
        slabs = {}     # (hp, hh) -> slab tile
        pairs = {}     # hp -> [pair tiles]

        def emit_scores_hh(hp, hh):
            p0 = hh * 64
            slab = slabp.tile([P, NKT, 384], VDT, tag="slab",
                              name=f"slab{hp}_{hh}")
            for kt in range(NKT):
                qlo = max(0, kt - 2)
                qhi = min(NQT - 1, kt)
                nq = (qhi - qlo + 1) * P
                ps = spool.tile([P, 384], F32, tag="spool",
                                name=f"st{hp}_{kt}_{hh}")
                nc.tensor.matmul(
                    ps[:, :nq],
                    KT[p0:p0 + 64, hp, kt * P:(kt + 1) * P],
                    QT[p0:p0 + 64, hp, qlo * P: qlo * P + nq],
                    start=True, stop=True)
                nc.scalar.activation(slab[:, kt, :nq], ps[:, :nq], Exp,
                                     bias=0.0, scale=float(scale))
                mis = mask_by_kt.get(kt, ())
                if len(mis) == 2 and mis[1][1] - mis[0][1] == 1:
                    # two adjacent masked blocks (kt==1): one 256-wide op
                    mi, qt = mis[0]
                    qoff = (qt - qlo) * P
                    nc.vector.tensor_tensor(
                        slab[:, kt, qoff:qoff + 2 * P],
                        slab[:, kt, qoff:qoff + 2 * P],
                        masks.rearrange("p a b -> p (a b)")
                             [:, mi * P:(mi + 2) * P], MUL)
                else:
                    for mi, qt in mis:
                        qoff = (qt - qlo) * P
                        nc.vector.tensor_tensor(
                            slab[:, kt, qoff:qoff + P],
                            slab[:, kt, qoff:qoff + P],
                            masks[:, mi, :], MUL)
            slabs[(hp, hh)] = slab

        def emit_av_hh(hp, hh):
            if hh == 0:
                pairs[hp] = [small.tile([P, P], PAIR_DT, tag="pair",
                                        name=f"pair{hp}_{i}")
                             for i in range(NQT)]
            pair = pairs[hp]
            h = 2 * hp + hh
            p0 = hh * 64
            slab = slabs.pop((hp, hh))
            pav = smallp.tile([P, NQT, D + 2], F32, tag="smallp",
                              name=f"pav{hp}_{hh}")
            for qt in range(NQT):
                for i, kt in enumerate(range(qt, qt + 3)):
                    qoff = (qt - max(0, kt - 2)) * P
                    nc.tensor.matmul(
                        pav[:, qt, :],
                        slab[:, kt, qoff:qoff + P],
                        VS[:, kt, h, :],
                        start=(i == 0), stop=(i == 2),
                        skip_group_check=True)
            rcp = small.tile([P, NQT], F32, tag="rcp")
            nc.vector.reciprocal(rcp, pav[:, :, D])
            for qt in range(NQT):
                nc.vector.tensor_scalar_mul(
                    pair[qt][:, p0:p0 + 64], pav[:, qt, 0:D],
                    rcp[:, qt:qt + 1])

        def emit_pair_fin(hp):
            # transpose head-pair outputs into c_in-major layout
            pair = pairs.pop(hp)
            for qg in range(2):
                pt = smallp.tile([P, 2 * P], PAIR_DT, tag="smallp",
                                 name=f"ptr{hp}_{qg}")
                for ql in range(2):
                    nc.tensor.transpose(
                        pt[:, ql * P:(ql + 1) * P], pair[qg * 2 + ql], identv)
                nc.vector.tensor_copy(
                    outT[:, hp, qg * 2 * P:(qg + 1) * 2 * P], pt)

        # 3-stage software pipeline at half-pair (head) granularity: the
        # AV/normalize for head (jt-1, hh) is emitted only once enough
        # independent work (K matmuls, scores) is queued ahead of it that
        # its EXPs have drained; transposes lag a full pair further.
        for jg in range(2):
            wts = wgroup(slice(C + jg * 512, C + (jg + 1) * 512),
                         fp8=QK_FP8)
            for jl in range(4):
                jt = jg * 4 + jl
                for half, hw in ((0, 512), (1, 256)):
                    ps = ps512.tile([P, 512], F32, tag="ps512")
                    if QK_FP8:
                        for g2 in range(4):
                            nc.tensor.matmul(
                                ps[:, :hw],
                                wts[:, 2 * g2:2 * g2 + 2,
                                    jl * P:(jl + 1) * P],
                                xT8[:, 2 * g2:2 * g2 + 2,
                                    half * 512: half * 512 + hw],
                                start=(g2 == 0), stop=(g2 == 3),
                                perf_mode=DR)
                    else:
                        for ct in range(C // P):
                            nc.tensor.matmul(
                                ps[:, :hw],
                                wts[:, ct, jl * P:(jl + 1) * P],
                                xT[:, ct, half * 512: half * 512 + hw],
                                start=(ct == 0), stop=(ct == C // P - 1))
                    if zero_bias:
                        nc.vector.tensor_copy(
                            KT[:, jt, half * 512: half * 512 + hw],
                            ps[:, :hw])
                    else:
                        nc.vector.tensor_scalar_add(
                            KT[:, jt, half * 512: half * 512 + hw],
                            ps[:, :hw], bqk[:, 8 + jt: 9 + jt])
                if jt >= 2:
                    emit_pair_fin(jt - 2)
                if jt >= 1:
                    emit_av_hh(jt - 1, 0)
                emit_scores_hh(jt, 0)
                if jt >= 1:
                    emit_av_hh(jt - 1, 1)
                emit_scores_hh(jt, 1)
        emit_pair_fin(6)
        emit_av_hh(7, 0)
        emit_av_hh(7, 1)
        emit_pair_fin(7)

        # ---------------- output projection
        for oc in range(2):
            wts = wpool.tile([P, 8, 512], VDT, tag="wchunk")
            nc.sync.dma_start(
                wts, wp[:, oc * 512:(oc + 1) * 512]
                    .rearrange("(hp p) j -> p hp j", p=P))
            for tb in range(NQT):
                ps = ps512.tile([P, 512], F32, tag="ps512")
                for hp in range(8):
                    nc.tensor.matmul(
                        ps,
                        outT[:, hp, tb * P:(tb + 1) * P],
                        wts[:, hp, :],
                        start=(hp == 0), stop=(hp == 7))
                ysb = yout.tile([P, 512], YDT, tag="ysb")
                if zero_bias:
                    nc.scalar.activation(ysb, ps, Ident, bias=0.0, scale=1.0)
                else:
                    nc.vector.tensor_tensor(
                        ysb, ps, bp_b[:, oc * 512:(oc + 1) * 512], ADD)
                nc.sync.dma_start(
                    y[tb * P:(tb + 1) * P, oc * 512:(oc + 1) * 512], ysb)

    nc.compile()
    return nc


def _get_module(zero_bias):
    if zero_bias not in _MODS:
        _MODS[zero_bias] = _build_module(zero_bias)
    return _MODS[zero_bias]


# ------------------------------------------------------------- host helpers
def _mask_tiles(chunk_start: int) -> np.ndarray:
    """[128, 9, 128]: multiplicative (1 valid / 0 invalid) tiles for
    MASK_BLOCKS."""
    out = np.zeros((P, 9, P), np.float32)
    kk = np.arange(P)[:, None]
    qq = np.arange(P)[None, :]

    def valid(kt, qt):
        key_abs = chunk_start - HALO + kt * P + kk
        q_abs = chunk_start + qt * P + qq
        return (key_abs <= q_abs) & (key_abs >= q_abs - WIN) & (key_abs >= 0)

    for i, (kt, qt) in enumerate(MASK_BLOCKS):
        out[:, i, :] = valid(kt, qt).astype(np.float32)
    return out


def _in_maps(x, W_attn, b_attn, W_proj, b_proj):
    import ml_dtypes
    f8 = np.dtype(ml_dtypes.float8_e4m3)
    sdt, vdt = _np_dt(SCORE_DT), _np_dt(VALUE_DT)
    wa = np.ascontiguousarray(np.asarray(W_attn, np.float32).astype(sdt))
    wpp = np.ascontiguousarray(np.asarray(W_proj, np.float32).astype(vdt))
    ba = np.ascontiguousarray(b_attn, np.float32)
    bpp = np.ascontiguousarray(b_proj, np.float32)
    if QK_FP8:
        w8 = np.ascontiguousarray(
            np.asarray(W_attn, np.float32)[:, :2 * C].astype(f8))
    maps = []
    for c in range(NCORES):
        b, k = divmod(c, NCORES // B)
        t0 = k * CHUNK
        xdt = sdt if SCORE_DT == "bf16" else np.dtype(np.float32)
        xhalo = np.zeros((TQ, C), xdt)
        lo = t0 - HALO
        src_lo = max(0, lo)
        xhalo[src_lo - lo:, :] = x[b, src_lo: t0 + CHUNK].astype(xdt)
        xt = np.ascontiguousarray(xhalo.T)            # [C, TQ]
        m = {
            "xh": xt,
            "wa": wa,
            "ba": ba,
            "wp": wpp,
            "bp": bpp,
            "mk": _mask_tiles(t0).astype(vdt),
        }
        if QK_FP8:
            m["xh8"] = np.ascontiguousarray(
                xt.astype(np.float32).astype(f8))
            m["w8"] = w8
        maps.append(m)
    return maps


def _run(inputs, trace=False, trace_kwargs=None):
    from concourse import bass_utils

    zero_bias = (not np.any(inputs["b_attn"])) and \
        (not np.any(inputs["b_proj"]))
    nc = _get_module(zero_bias)
    maps = _in_maps(**inputs)
    res = bass_utils.run_bass_kernel_spmd(
        nc, maps, core_ids=list(range(NCORES)),
        trace=trace, **(trace_kwargs or {}))
    out = np.empty((B, T, C), np.float32)
    for c in range(NCORES):
        b, k = divmod(c, NCORES // B)
        out[b, k * CHUNK:(k + 1) * CHUNK] = \
            np.asarray(res.results[c]["y"], np.float32)
    return out, res


def kernel(x, W_attn, b_attn, W_proj, b_proj):
    inputs = dict(x=np.asarray(x, np.float32), W_attn=W_attn, b_attn=b_attn,
                  W_proj=W_proj, b_proj=b_proj)
    out, _ = _run(inputs)
    return out


# revision 60
# speedup vs baseline: 13645.8062x; 1.0885x over previous
"""Trainium2 Bass kernel for local (windowed causal) self-attention.

Problem: B=2, T=2048, C=1024, 16 heads x 64 dim, local window 256.
Sharding: T-sharding. 8 cores = 2 batches x 4 chunks of 512 tokens.
Each core receives its 512-token chunk plus a 256-token left halo of x
(pre-transposed to [C, TQ] on the host, zero-padded for chunk 0),
computes QKV / banded attention / output projection for its own rows,
and writes a disjoint [512, 1024] slice of the output. No collectives;
the host concatenates the 8 slices.

Self-contained: hardcodes all shapes; no reads of /root/problem/*.
"""

import os

os.environ.setdefault("MYCRO_LOCAL_CACHE", "1")

import numpy as np

# ---------------------------------------------------------------- constants
B, T, C = 2, 2048, 1024
H, D = 16, 64
WIN = 256                      # local attention context
NCORES = 8
CHUNK = 512                    # queries per core
HALO = 256                     # left halo (== WIN)
TQ = CHUNK + HALO              # 768 x rows per core
P = 128

NQT = CHUNK // P               # 4 query tiles per core
NKT = TQ // P                  # 6 key tiles per core

# Masked (kt, qt) blocks, all handled as multiplicative 0/1 masks on the
# vector engine post-exp (safe: halo x is host-zeroed, so even "invalid"
# scores are exactly 0 and exp never overflows). kt 1..3 have two masked
# blocks; they are adjacent in the slab for kt 1 only.
MASK_BLOCKS = [(0, 0), (1, 0), (1, 1), (2, 0), (2, 2),
               (3, 1), (3, 3), (4, 2), (5, 3)]

# Matmul operand dtypes: "bf16" or "f32r".
SCORE_DT = os.environ.get("KERNEL_SCORE_DT", "bf16")
VALUE_DT = os.environ.get("KERNEL_VALUE_DT", "bf16")
Y_BF16 = os.environ.get("KERNEL_Y_BF16", "1") == "1"
N_WARM = int(os.environ.get("KERNEL_WARM", "8"))
# Q/K projections in fp8e4m3 with DoubleRow (2x PE throughput). Softmax
# normalization absorbs the score-path quantization (rel err ~1.4e-2 vs the
# 2e-2 gate); V stays bf16 (V errors pass straight through to the output).
QK_FP8 = os.environ.get("KERNEL_QK_FP8", "1") == "1"

_MODS = {}                     # cached compiled Bass modules


def _np_dt(name):
    if name == "bf16":
        import ml_dtypes
        return np.dtype(ml_dtypes.bfloat16)
    return np.dtype(np.float32)


# ------------------------------------------------------------- bass builder
def _build_module(zero_bias):
    import concourse.bacc as bacc
    import concourse.mybir as mybir
    import concourse.tile as tile
    from concourse.masks import make_identity
    from contextlib import ExitStack

    F32 = mybir.dt.float32
    BF16 = mybir.dt.bfloat16
    SDT = BF16 if SCORE_DT == "bf16" else mybir.dt.float32r
    VDT = BF16 if VALUE_DT == "bf16" else mybir.dt.float32r
    YDT = BF16 if Y_BF16 else F32

    nc = bacc.Bacc(
        "TRN2",
        target_bir_lowering=False,
        debug=False,
        enable_asserts=False,
        num_devices=NCORES,
    )

    XDT = SDT if SCORE_DT == "bf16" else F32
    F8 = mybir.dt.float8e4
    # x^T is prepared on the host: [C, TQ]
    xh = nc.dram_tensor("xh", [C, TQ], XDT, kind="ExternalInput").ap()
    wa = nc.dram_tensor("wa", [C, 3 * C], SDT, kind="ExternalInput").ap()
    if QK_FP8:
        # fp8 copies of x^T and W_attn[:, :2C] for the Q/K projections
        xh8 = nc.dram_tensor("xh8", [C, TQ], F8, kind="ExternalInput").ap()
        w8 = nc.dram_tensor("w8", [C, 2 * C], F8, kind="ExternalInput").ap()
    ba = nc.dram_tensor("ba", [3 * C], F32, kind="ExternalInput").ap()
    wp = nc.dram_tensor("wp", [C, C], VDT, kind="ExternalInput").ap()
    bp = nc.dram_tensor("bp", [C], F32, kind="ExternalInput").ap()
    # multiplicative (0/1) mask tiles for MASK_BLOCKS: [128k, 9, 128q]
    mk = nc.dram_tensor("mk", [P, 9, P], VDT, kind="ExternalInput").ap()
    y = nc.dram_tensor("y", [CHUNK, C], YDT, kind="ExternalOutput").ap()

    Exp = mybir.ActivationFunctionType.Exp
    Ident = mybir.ActivationFunctionType.Identity
    ADD = mybir.AluOpType.add
    MUL = mybir.AluOpType.mult

    with tile.TileContext(nc) as tc, ExitStack() as ctx:
        const = ctx.enter_context(tc.tile_pool(name="const", bufs=1))
        xload = ctx.enter_context(tc.tile_pool(name="xload", bufs=2))
        big = ctx.enter_context(tc.tile_pool(name="big", bufs=1))
        wpool = ctx.enter_context(tc.tile_pool(name="wpool", bufs=3))
        slabp = ctx.enter_context(tc.tile_pool(name="slabp", bufs=4))
        small = ctx.enter_context(tc.tile_pool(name="small", bufs=16))
        yout = ctx.enter_context(tc.tile_pool(name="yout", bufs=4))
        # PSUM: 8 banks of 2KB. ps512 x3 (QKV/proj + K), spool x3 (scores),
        # smallp x2 (AV accum + pair transposes, interleaved allocations).
        ps512 = ctx.enter_context(tc.tile_pool(name="ps512", bufs=3, space="PSUM"))
        spool = ctx.enter_context(tc.tile_pool(name="spool", bufs=3, space="PSUM"))
        smallp = ctx.enter_context(tc.tile_pool(name="smallp", bufs=2, space="PSUM"))

        # ---------------- constants
        ident = const.tile([P, P], F32)
        make_identity(nc, ident)
        if VALUE_DT == "bf16":
            identv = const.tile([P, P], BF16)
            make_identity(nc, identv)
            PAIR_DT = BF16
        else:
            identv = ident
            PAIR_DT = F32

        if not zero_bias:
            bqk = const.tile([P, 16], F32)      # b_attn[:2048] as [128, jt]
            with nc.allow_non_contiguous_dma(reason="tiny bias rearrange"):
                nc.sync.dma_start(
                    bqk, ba[: 2 * C].rearrange("(j p) -> p j", p=P))
            bv_row = xload.tile([1, C], F32, tag="brow")
            nc.sync.dma_start(bv_row, ba[None, 2 * C:])
            bv_b = const.tile([P, C], F32)
            nc.gpsimd.partition_broadcast(bv_b, bv_row)
            bp_row = xload.tile([1, C], F32, tag="brow")
            nc.sync.dma_start(bp_row, bp[None, :])
            bp_b = const.tile([P, C], F32)
            nc.gpsimd.partition_broadcast(bp_b, bp_row)

        # PE warm-up: dense dummy matmuls while the first DMAs land, so the
        # p-state ramp reaches full clock before real matmuls start.
        warm = const.tile([P, 512], BF16)
        nc.vector.memset(warm, 0.0)
        for wi in range(N_WARM):
            wps = ps512.tile([P, 512], F32, tag="ps512", name=f"wps{wi}")
            nc.tensor.matmul(wps, warm[:, :P], warm, start=True, stop=True)

        # ---------------- high-priority DMAs: Q weights (group 0) + own x^T
        # One DMA per logical group (rearranged AP): dma_start issue time on
        # the sync queue is ~0.6us each, so merging is critical for the head.
        xT = big.tile([P, C // P, TQ], SDT, tag="xT")

        def wgroup(src_cols, split=False, fp8=False):
            dt_ = F8 if fp8 else SDT
            base = w8 if fp8 else wa
            wt = wpool.tile([P, C // P, 512], dt_, tag="wchunk")
            src = base[:, src_cols].rearrange("(ct p) j -> p ct j", p=P)
            if split:
                nc.sync.dma_start(wt[:, :4], src[:, :4])
                nc.sync.dma_start(wt[:, 4:], src[:, 4:])
            else:
                nc.sync.dma_start(wt, src)
            return wt

        w_q0 = wgroup(slice(0, 512), split=True, fp8=QK_FP8)
        if QK_FP8:
            xT8 = big.tile([P, C // P, TQ], F8, tag="xT8")
            x8src = xh8.rearrange("(ct p) t -> p ct t", p=P)
            nc.sync.dma_start(xT8[:, :, HALO:TQ], x8src[:, :, HALO:TQ])
            # bf16 x^T next: the V projection needs it right after the
            # (short, fp8) Q phase
            xsrc = xh.rearrange("(ct p) t -> p ct t", p=P)
            nc.sync.dma_start(xT[:, :4], xsrc[:, :4])
            nc.sync.dma_start(xT[:, 4:], xsrc[:, 4:])
        else:
            xsrc = xh[:, HALO:TQ].rearrange("(ct p) t -> p ct t", p=P)
            nc.sync.dma_start(xT[:, :4, HALO:TQ], xsrc[:, :4])
            nc.sync.dma_start(xT[:, 4:, HALO:TQ], xsrc[:, 4:])

        masks = const.tile([P, 9, P], VDT)
        nc.sync.dma_start(masks, mk)

        w_q1 = wgroup(slice(512, 1024), fp8=QK_FP8)
        if QK_FP8:
            nc.sync.dma_start(xT8[:, :, 0:HALO], x8src[:, :, 0:HALO])
        else:
            nc.sync.dma_start(                   # halo columns of x^T
                xT[:, :, 0:HALO],
                xh[:, 0:HALO].rearrange("(ct p) t -> p ct t", p=P))
        w_q = [w_q0, w_q1]

        # ---------------- QKV
        # Q^T [128j, jt, 512t(own)]  /  K^T [128j, jt, 768t]
        QT = big.tile([P, 8, CHUNK], SDT, tag="QT")
        KT = big.tile([P, 8, TQ], SDT, tag="KT")
        # V natural + ones columns: [128t, tt, head, D+2]
        VS = big.tile([P, NKT, H, D + 2], VDT, tag="VS")
        ones_h = const.tile([P, NKT * H], F32)
        nc.gpsimd.memset(ones_h, 1.0)
        nc.vector.tensor_copy(
            VS[:, :, :, D], ones_h.rearrange("p (t h) -> p t h", h=H))
        nc.vector.tensor_copy(
            VS[:, :, :, D + 1], ones_h.rearrange("p (t h) -> p t h", h=H))

        DR = mybir.MatmulPerfMode.DoubleRow

        # --- Q part: lhsT = W_attn[:, j] tile, rhs = xT own rows
        for jg in range(2):                       # 2 groups of 4 j-tiles
            wts = w_q[jg]
            for jl in range(4):
                jt = jg * 4 + jl
                ps = ps512.tile([P, CHUNK], F32, tag="ps512")
                if QK_FP8:
                    for g2 in range(4):           # 4 DoubleRow k-pairs
                        nc.tensor.matmul(
                            ps,
                            wts[:, 2 * g2:2 * g2 + 2, jl * P:(jl + 1) * P],
                            xT8[:, 2 * g2:2 * g2 + 2, HALO:TQ],
                            start=(g2 == 0), stop=(g2 == 3), perf_mode=DR)
                else:
                    for ct in range(C // P):
                        nc.tensor.matmul(
                            ps,
                            wts[:, ct, jl * P:(jl + 1) * P],
                            xT[:, ct, HALO:TQ],
                            start=(ct == 0), stop=(ct == C // P - 1))
                nc.scalar.activation(
                    QT[:, jt, :], ps, Ident, scale=1.0,
                    bias=0.0 if zero_bias else bqk[:, jt:jt + 1])

        # --- V part: lhsT = xT tile, rhs = W_attn[:, 2048+...]
        for vc in range(2):
            wts = wgroup(slice(2 * C + vc * 512, 2 * C + (vc + 1) * 512))
            for tt in range(NKT):
                ps = ps512.tile([P, 512], F32, tag="ps512")
                for ct in range(C // P):
                    nc.tensor.matmul(
                        ps,
                        xT[:, ct, tt * P:(tt + 1) * P],
                        wts[:, ct, :],
                        start=(ct == 0), stop=(ct == C // P - 1))
                if zero_bias:
                    nc.scalar.activation(
                        VS[:, tt, vc * 8:(vc + 1) * 8, 0:D],
                        ps.rearrange("p (h d) -> p h d", d=D),
                        Ident, bias=0.0, scale=1.0)
                else:
                    nc.vector.tensor_tensor(
                        VS[:, tt, vc * 8:(vc + 1) * 8, 0:D],
                        ps.rearrange("p (h d) -> p h d", d=D),
                        bv_b[:, vc * 512:(vc + 1) * 512]
                            .rearrange("p (h d) -> p h d", d=D),
                        ADD)

        # --- K part, with attention head-pairs interleaved so the PE
        # stream stays dense and engines overlap across phases.
        outT = big.tile([P, 8, CHUNK], VDT, tag="outT")  # [c_pair, hp, t]
        scale = 1.0 / np.sqrt(D)

        mask_by_kt = {}
        for i, (kt, qt) in enumerate(MASK_BLOCKS):
            mask_by_kt.setdefault(kt, []).append((i, qt))
        slabs = {}     # (hp, hh) -> slab tile
        pairs = {}     # hp -> [pair tiles]

        def emit_scores_hh(hp, hh):
            p0 = hh * 64
            slab = slabp.tile([P, NKT, 384], VDT, tag="slab",
                              name=f"slab{hp}_{hh}")
            for kt in range(NKT):
                qlo = max(0, kt - 2)
                qhi = min(NQT - 1, kt)
                nq = (qhi - qlo + 1) * P
                ps = spool.tile([P, 384], F32, tag="spool",
                                name=f"st{hp}_{kt}_{hh}")
                nc.tensor.matmul(
                    ps[:, :nq],
                    KT[p0:p0 + 64, hp, kt * P:(kt + 1) * P],
                    QT[p0:p0 + 64, hp, qlo * P: qlo * P + nq],
                    start=True, stop=True)
                nc.scalar.activation(slab[:, kt, :nq], ps[:, :nq], Exp,
                                     bias=0.0, scale=float(scale))
                mis = mask_by_kt.get(kt, ())
                if len(mis) == 2 and mis[1][1] - mis[0][1] == 1:
                    # two adjacent masked blocks (kt==1): one 256-wide op
                    mi, qt = mis[0]
                    qoff = (qt - qlo) * P
                    nc.vector.tensor_tensor(
                        slab[:, kt, qoff:qoff + 2 * P],
                        slab[:, kt, qoff:qoff + 2 * P],
                        masks.rearrange("p a b -> p (a b)")
                             [:, mi * P:(mi + 2) * P], MUL)
                else:
                    for mi, qt in mis:
                        qoff = (qt - qlo) * P
                        nc.vector.tensor_tensor(
                            slab[:, kt, qoff:qoff + P],
                            slab[:, kt, qoff:qoff + P],
                            masks[:, mi, :], MUL)
            slabs[(hp, hh)] = slab

        def emit_av_hh(hp, hh):
            if hh == 0:
                pairs[hp] = [small.tile([P, P], PAIR_DT, tag="pair",
                                        name=f"pair{hp}_{i}")
                             for i in range(NQT)]
            pair = pairs[hp]
            h = 2 * hp + hh
            p0 = hh * 64
            slab = slabs.pop((hp, hh))
            pav = smallp.tile([P, NQT, D + 2], F32, tag="smallp",
                              name=f"pav{hp}_{hh}")
            for qt in range(NQT):
                for i, kt in enumerate(range(qt, qt + 3)):
                    qoff = (qt - max(0, kt - 2)) * P
                    nc.tensor.matmul(
                        pav[:, qt, :],
                        slab[:, kt, qoff:qoff + P],
                        VS[:, kt, h, :],
                        start=(i == 0), stop=(i == 2),
                        skip_group_check=True)
            rcp = small.tile([P, NQT], F32, tag="rcp")
            nc.vector.reciprocal(rcp, pav[:, :, D])
            for qt in range(NQT):
                nc.vector.tensor_scalar_mul(
                    pair[qt][:, p0:p0 + 64], pav[:, qt, 0:D],
                    rcp[:, qt:qt + 1])

        def emit_pair_fin(hp):
            # transpose head-pair outputs into c_in-major layout
            pair = pairs.pop(hp)
            for qg in range(2):
                pt = smallp.tile([P, 2 * P], PAIR_DT, tag="smallp",
                                 name=f"ptr{hp}_{qg}")
                for ql in range(2):
                    nc.tensor.transpose(
                        pt[:, ql * P:(ql + 1) * P], pair[qg * 2 + ql], identv)
                nc.vector.tensor_copy(
                    outT[:, hp, qg * 2 * P:(qg + 1) * 2 * P], pt)

        # 3-stage software pipeline at half-pair (head) granularity: the
        # AV/normalize for head (jt-1, hh) is emitted only once enough
        # independent work (K matmuls, scores) is queued ahead of it that
        # its EXPs have drained; transposes lag a full pair further.
        # K stays bf16 (verified on clean-clock traces: fp8 K removes the PE
        # work that productively fills the attention phase's cross-engine
        # latency bubbles, stretching the phase by what the matmuls saved).
        for jg in range(2):
            wts = wgroup(slice(C + jg * 512, C + (jg + 1) * 512))
            for jl in range(4):
                jt = jg * 4 + jl
                for half, hw in ((0, 512), (1, 256)):
                    ps = ps512.tile([P, 512], F32, tag="ps512")
                    for ct in range(C // P):
                        nc.tensor.matmul(
                            ps[:, :hw],
                            wts[:, ct, jl * P:(jl + 1) * P],
                            xT[:, ct, half * 512: half * 512 + hw],
                            start=(ct == 0), stop=(ct == C // P - 1))
                    if zero_bias:
                        nc.vector.tensor_copy(
                            KT[:, jt, half * 512: half * 512 + hw],
                            ps[:, :hw])
                    else:
                        nc.vector.tensor_scalar_add(
                            KT[:, jt, half * 512: half * 512 + hw],
                            ps[:, :hw], bqk[:, 8 + jt: 9 + jt])
                if jt >= 2:
                    emit_pair_fin(jt - 2)
                if jt >= 1:
                    emit_av_hh(jt - 1, 0)
                emit_scores_hh(jt, 0)
                if jt >= 1:
                    emit_av_hh(jt - 1, 1)
                emit_scores_hh(jt, 1)
        emit_pair_fin(6)
        emit_av_hh(7, 0)
        emit_av_hh(7, 1)
        emit_pair_fin(7)

        # ---------------- output projection
        for oc in range(2):
            wts = wpool.tile([P, 8, 512], VDT, tag="wchunk")
            nc.sync.dma_start(
                wts, wp[:, oc * 512:(oc + 1) * 512]
                    .rearrange("(hp p) j -> p hp j", p=P))
            for tb in range(NQT):
                ps = ps512.tile([P, 512], F32, tag="ps512")
                for hp in range(8):
                    nc.tensor.matmul(
                        ps,
                        outT[:, hp, tb * P:(tb + 1) * P],
                        wts[:, hp, :],
                        start=(hp == 0), stop=(hp == 7))
                ysb = yout.tile([P, 512], YDT, tag="ysb")
                if zero_bias:
                    nc.scalar.activation(ysb, ps, Ident, bias=0.0, scale=1.0)
                else:
                    nc.vector.tensor_tensor(
                        ysb, ps, bp_b[:, oc * 512:(oc + 1) * 512], ADD)
                nc.sync.dma_start(
                    y[tb * P:(tb + 1) * P, oc * 512:(oc + 1) * 512], ysb)

    nc.compile()
    return nc


def _get_module(zero_bias):
    if zero_bias not in _MODS:
        _MODS[zero_bias] = _build_module(zero_bias)
    return _MODS[zero_bias]


# ------------------------------------------------------------- host helpers
def _mask_tiles(chunk_start: int) -> np.ndarray:
    """[128, 9, 128]: multiplicative (1 valid / 0 invalid) tiles for
    MASK_BLOCKS."""
    out = np.zeros((P, 9, P), np.float32)
    kk = np.arange(P)[:, None]
    qq = np.arange(P)[None, :]

    def valid(kt, qt):
        key_abs = chunk_start - HALO + kt * P + kk
        q_abs = chunk_start + qt * P + qq
        return (key_abs <= q_abs) & (key_abs >= q_abs - WIN) & (key_abs >= 0)

    for i, (kt, qt) in enumerate(MASK_BLOCKS):
        out[:, i, :] = valid(kt, qt).astype(np.float32)
    return out


def _in_maps(x, W_attn, b_attn, W_proj, b_proj):
    import ml_dtypes
    f8 = np.dtype(ml_dtypes.float8_e4m3)
    sdt, vdt = _np_dt(SCORE_DT), _np_dt(VALUE_DT)
    wa = np.ascontiguousarray(np.asarray(W_attn, np.float32).astype(sdt))
    wpp = np.ascontiguousarray(np.asarray(W_proj, np.float32).astype(vdt))
    ba = np.ascontiguousarray(b_attn, np.float32)
    bpp = np.ascontiguousarray(b_proj, np.float32)
    if QK_FP8:
        w8 = np.ascontiguousarray(
            np.asarray(W_attn, np.float32)[:, :2 * C].astype(f8))
    maps = []
    for c in range(NCORES):
        b, k = divmod(c, NCORES // B)
        t0 = k * CHUNK
        xdt = sdt if SCORE_DT == "bf16" else np.dtype(np.float32)
        xhalo = np.zeros((TQ, C), xdt)
        lo = t0 - HALO
        src_lo = max(0, lo)
        xhalo[src_lo - lo:, :] = x[b, src_lo: t0 + CHUNK].astype(xdt)
        xt = np.ascontiguousarray(xhalo.T)            # [C, TQ]
        m = {
            "xh": xt,
            "wa": wa,
            "ba": ba,
            "wp": wpp,
            "bp": bpp,
            "mk": _mask_tiles(t0).astype(vdt),
        }
        if QK_FP8:
            m["xh8"] = np.ascontiguousarray(
                xt.astype(np.float32).astype(f8))
            m["w8"] = w8
        maps.append(m)
    return maps


def _run(inputs, trace=False, trace_kwargs=None):
    from concourse import bass_utils

    zero_bias = (not np.any(inputs["b_attn"])) and \
        (not np.any(inputs["b_proj"]))
    nc = _get_module(zero_bias)
    maps = _in_maps(**inputs)
    res = bass_utils.run_bass_kernel_spmd(
        nc, maps, core_ids=list(range(NCORES)),
        trace=trace, **(trace_kwargs or {}))
    out = np.empty((B, T, C), np.float32)
    for c in range(NCORES):
        b, k = divmod(c, NCORES // B)
        out[b, k * CHUNK:(k + 1) * CHUNK] = \
            np.asarray(res.results[c]["y"], np.float32)
    return out, res


def kernel(x, W_attn, b_attn, W_proj, b_proj):
    inputs = dict(x=np.asarray(x, np.float32), W_attn=W_attn, b_attn=b_attn,
                  W_proj=W_proj, b_proj=b_proj)
    out, _ = _run(inputs)
    return out
